# revision 62
# baseline (speedup 1.0000x reference)
"""AssetGCN Trainium2 kernel: 8-core data-parallel over asset groups.

Global problem: G=128 groups x A=100 assets, WIN=10, FD=16, H=128.
Per core: 16 groups (1600 nodes). No collectives needed (fully group-parallel).

Per-chunk (4 groups) pipeline, all on-device:
  adjacency via matmuls batched 4-groups-wide (row sums with a ones vector,
  cov = r@r.T - s(x)s/W, diagonal scalings as K=1 outer-product matmuls
  folded elementwise), 3 GCN layers in bf16 with per-layer 4-group-batched
  PSUM banks and single evictions, then two 1x3 convs along the hidden axis
  as bf16 matmuls: conv1 uses 32-row windowed banded stationaries (small
  LDWEIGHTS), conv2 accumulates 128 column-padded [128,128] patterns into
  one PSUM bank per chunk. GCN work for chunk ch+1 is interleaved into the
  conv m-loop of chunk ch to keep the PE queue dense.
"""

import numpy as np

NCORES = 8
A = 100
WIN = 10
FD = 16
H = 128
F160 = WIN * FD
G_PER_CORE = 16
NODES = G_PER_CORE * A          # 1600 per core
CHUNK = 400                     # nodes per conv chunk (4 groups)
NCHUNK = NODES // CHUNK         # 4
GPC = CHUNK // A                # groups per chunk: 4


def _host_consts(inputs):
    """Precompute replicated weight/const arrays (numpy, shared by all cores)."""
    import ml_dtypes

    f32 = np.float32
    bf16 = np.dtype(ml_dtypes.bfloat16)
    W1 = np.ascontiguousarray(inputs["W1"], f32)          # [160,128]
    W2 = np.ascontiguousarray(inputs["W2"], f32)          # [128,128]
    W3 = np.ascontiguousarray(inputs["W3"], f32)          # [128,128]
    cw1 = np.asarray(inputs["cw1"], f32)                  # [128,1,1,3]
    cw2 = np.asarray(inputs["cw2"], f32)                  # [1,128,1,3]
    cw1r = np.ascontiguousarray(cw1[:, 0, 0, :].T)        # [3,128] rows t
    cw2m = cw2[0, :, 0, :]                                # [128,3] cols k

    # conv1 stationaries: 34 banded K=32 window patterns, replicated down
    # the partition dim so windows at bases 0/32/64 all have a copy; the
    # s=3 window (base 96 is illegal for matmul APs) reads a separate
    # base-0 copy of h3 rows 96..127.
    # u[c,n,m] = sum_t cw1r[t,c] * h[m-1+t, n]; pattern row r (within its
    # 32-row window) multiplies h3 window row r.
    pats = np.zeros((34, 32, H), f32)
    for i in range(1, 31):              # interior offsets, block i-1
        pats[i - 1, i - 1] = cw1r[0]
        pats[i - 1, i] = cw1r[1]
        pats[i - 1, i + 1] = cw1r[2]
    pats[30, 0] = cw1r[1]               # m==0 / i==0 low part
    pats[30, 1] = cw1r[2]
    pats[31, 30] = cw1r[0]              # m==127 / i==31 low part
    pats[31, 31] = cw1r[1]
    pats[32, 31] = cw1r[0]              # i==0 edge: row 31 of prev window
    pats[33, 0] = cw1r[2]               # i==31 edge: row 0 of next window
    cw1p = pats.transpose(1, 0, 2).reshape(32, 34 * H)
    cw1p = np.tile(cw1p, (4, 1))        # [128, 34*128]

    # conv2 stationaries: one [128,128] column-padded pattern per position m:
    # column j of pattern m = cw2[:, k] where k = m - j + 1 (|j - m| <= 1).
    c2 = np.zeros((H, H, H), f32)       # [c, m, j]
    for m in range(H):
        for dj, k in ((-1, 2), (0, 1), (1, 0)):
            j = m + dj
            if 0 <= j < H:
                c2[:, m, j] = cw2m[:, k]
    cw2full = c2.reshape(H, H * H)

    eye = np.eye(A, dtype=f32)
    consts = {
        "eye1_4": np.ascontiguousarray(np.tile(eye + 1.0, (1, GPC))),
        "eyeH": np.eye(H, dtype=f32),
        "W1a": np.ascontiguousarray(W1[:128]).astype(bf16),
        "W1b": np.ascontiguousarray(W1[128:]).astype(bf16),
        "W2": W2.astype(bf16),
        "W3": W3.astype(bf16),
    }
    consts["cw1p"] = np.ascontiguousarray(cw1p).astype(bf16)
    # full [128,128] row-banded patterns for positions whose 32-row window
    # would need an (illegal) base-32/64 matmul AP
    c1 = np.zeros((H, H, H), f32)          # [m, r, c]
    for m in range(H):
        for t in range(3):
            r = m + t - 1
            if 0 <= r < H:
                c1[m, r, :] = cw1r[t]
    cw1full = c1.transpose(1, 0, 2).reshape(H, H * H)
    # of the full-pattern bank only m=31 (col 31), 32..95, and m=96
    # (col 96) are used; trim to those. DMAs issue in dict order, which
    # matches first use in the conv m-loop so arrivals lead demand.
    def _cw1cols(a, b):
        return np.ascontiguousarray(cw1full[:, a * H:b * H]).astype(bf16)

    def _cw2q(q):
        return np.ascontiguousarray(
            cw2full[:, q * 32 * H:(q + 1) * 32 * H]).astype(bf16)

    consts["cw2q0"] = _cw2q(0)
    consts["cw1e31"] = _cw1cols(31, 32)
    consts["cw1f1"] = _cw1cols(32, 64)
    consts["cw2q1"] = _cw2q(1)
    consts["cw1f2"] = _cw1cols(64, 96)
    consts["cw2q2"] = _cw2q(2)
    consts["cw1e96"] = _cw1cols(96, 97)
    consts["cw2q3"] = _cw2q(3)
    meta = {
        "b1": np.asarray(inputs["b1"], f32),
        "b2": np.asarray(inputs["b2"], f32),
        "b3": np.asarray(inputs["b3"], f32),
        "cb1": np.asarray(inputs["cb1"], f32),
        "cb2": float(np.asarray(inputs["cb2"], f32).reshape(-1)[0]),
    }
    if meta["b1"].any():
        consts["b1row"] = np.ascontiguousarray(meta["b1"][None, :]).astype(bf16)
    if meta["b2"].any():
        consts["b2row"] = np.ascontiguousarray(meta["b2"][None, :]).astype(bf16)
    if meta["b3"].any():
        consts["b3col"] = np.ascontiguousarray(meta["b3"][:, None])
    if meta["cb1"].any():
        consts["cb1col"] = np.ascontiguousarray(meta["cb1"][:, None])
    return consts, meta


_NO_SPLIT = {
    "InstEventSemaphore",
    "InstUnconditionalBranch",
    "InstRegisterMove",
    "InstNoOp",
}


def _split_matmul_waits(nc, mybir, max_waits=1):
    """The TPB ISA carries one sync-wait slot per instruction and walrus
    rejects instructions with more; hoist extras onto same-engine NoOps."""
    ctr = 0
    for blk in nc.m.functions[0].blocks:
        out, changed = [], False
        for inst in blk.instructions:
            si = inst.sync_info
            if (
                type(inst).__name__ not in _NO_SPLIT
                and si is not None
                and si.on_wait
                and len(si.on_wait) > max_waits
            ):
                waits = list(si.on_wait)
                extra, keep = waits[:-max_waits], waits[-max_waits:]
                for w in extra:
                    ctr += 1
                    nop = mybir.InstNoOp(name=f"mmw-{ctr}", ins=[], outs=[])
                    nop.engine = inst.engine
                    nop.sync_info = mybir.SyncInfo(on_wait=[w], on_update=[])
                    out.append(nop)
                inst.sync_info = mybir.SyncInfo(
                    on_wait=keep, on_update=list(si.on_update)
                )
                changed = True
            out.append(inst)
        if changed:
            blk.instructions = out
    return ctr


def _build(consts, meta):
    import ml_dtypes
    import concourse.bass as bass
    import concourse.tile as tile
    from concourse import bacc, mybir

    F32 = mybir.dt.float32
    BF16 = mybir.dt.bfloat16
    AF = mybir.ActivationFunctionType
    np_bf16 = np.dtype(ml_dtypes.bfloat16)

    def _mt(v):
        return BF16 if v.dtype == np_bf16 else F32

    nc = bacc.Bacc()

    x_e = nc.declare_dram_parameter("x", [NODES, WIN, FD], F32, isOutput=False)
    out_e = nc.declare_dram_parameter("out", [NODES, H], F32, isOutput=True)
    ce = {
        k: nc.declare_dram_parameter(k, list(v.shape), _mt(v), isOutput=False)
        for k, v in consts.items()
    }

    with tile.TileContext(nc) as tc:
        with (
            tc.tile_pool(name="singles", bufs=1) as singles,
            tc.tile_pool(name="work", bufs=2) as work,
            tc.tile_pool(name="h3pool", bufs=2) as h3pool,
            tc.tile_pool(name="convsb", bufs=4) as convsb,
            tc.tile_pool(name="ps", bufs=2, space="PSUM") as ps,
            tc.tile_pool(name="psy", bufs=4, space="PSUM") as psy,
            tc.tile_pool(name="pso", bufs=1, space="PSUM") as pso,
            tc.tile_pool(name="pst", bufs=1, space="PSUM") as pst,
        ):
            # ---- chunk 0 input prefetch (ahead of the const stream so
            # gcn(0) — whose output gates the first conv — starts at once)
            ga0 = 0
            feats4_0 = work.tile([A, GPC * F160], F32, tag="feats4")
            nc.sync.dma_start(
                out=feats4_0.rearrange("a (g m) -> a g m", g=GPC),
                in_=x_e[ga0:ga0 + CHUNK].rearrange(
                    "(g a) w f -> a g (w f)", g=GPC
                ),
            )
            rT4_0 = work.tile([WIN, CHUNK], F32, tag="rT4")
            nc.sync.dma_start(
                out=rT4_0,
                in_=x_e[ga0:ga0 + CHUNK, :, FD - 1].rearrange("n w -> w n"),
            )

            # ---- load constants (alternate the two HWDGE queues) ----
            cs = {}
            for n, (k, v) in enumerate(consts.items()):
                t = singles.tile(list(v.shape), _mt(v), tag=f"c_{k}")
                eng = nc.sync if n % 2 == 0 else nc.scalar
                eng.dma_start(out=t, in_=ce[k][:])
                cs[k] = t
            ones10b = singles.tile([WIN, 1], BF16, tag="ones10b")
            nc.vector.memset(ones10b, 1.0)
            onesA = singles.tile([A, 1], F32, tag="onesA")
            nc.vector.memset(onesA, 1.0)
            ones1A = singles.tile([1, A], BF16, tag="ones1A")
            nc.vector.memset(ones1A, 1.0)

            def gcn_chunk(ch, h3t_dst, xpair=None):
                """Generator: builds 4 groups' adjacency + 3 GCN layers,
                batched 4-wide along the free axis; writes h3^T [128, 400]
                (hidden on partitions) into h3t_dst. Yields between
                dependency-separated emission steps so the caller can
                interleave conv matmuls into the PE queue."""
                if xpair is not None:
                    feats4, rT4 = xpair
                else:
                    ga = ch * CHUNK
                    feats4 = work.tile([A, GPC * F160], F32, tag="feats4")
                    nc.sync.dma_start(
                        out=feats4.rearrange("a (g m) -> a g m", g=GPC),
                        in_=x_e[ga:ga + CHUNK].rearrange(
                            "(g a) w f -> a g (w f)", g=GPC
                        ),
                    )
                    rT4 = work.tile([WIN, CHUNK], F32, tag="rT4")
                    nc.sync.dma_start(
                        out=rT4,
                        in_=x_e[ga:ga + CHUNK, :, FD - 1].rearrange("n w -> w n"),
                    )
                rT4b = work.tile([WIN, CHUNK], BF16, tag="rT4b")
                nc.vector.tensor_copy(rT4b, rT4)
                rT2b = work.tile([WIN, CHUNK], BF16, tag="rT2b")
                nc.vector.tensor_mul(rT2b, rT4, rT4)
                ps_row1 = ps.tile([1, CHUNK], F32, tag="gps")
                nc.tensor.matmul(ps_row1, ones10b, rT4b, start=True, stop=True)
                ps_row2 = ps.tile([1, CHUNK], F32, tag="gps")
                nc.tensor.matmul(ps_row2, ones10b, rT2b, start=True, stop=True)
                yield

                sT4 = work.tile([1, CHUNK], F32, tag="sT4")
                nc.scalar.activation(sT4, ps_row1, AF.Copy)
                sTn4 = work.tile([1, CHUNK], F32, tag="sTn4")
                nc.scalar.activation(sTn4, ps_row1, AF.Copy, scale=-1.0 / WIN)
                sumsqT4 = work.tile([1, CHUNK], F32, tag="sumsqT4")
                nc.scalar.activation(sumsqT4, ps_row2, AF.Copy)
                yield

                # cov = r@r.T - (1/W) s (x) s, per group into one bank
                ps_cov4 = ps.tile([A, CHUNK], F32, tag="gps")
                for g in range(GPC):
                    gs = slice(g * A, (g + 1) * A)
                    nc.tensor.matmul(
                        ps_cov4[:, gs], rT4[:, gs], rT4[:, gs],
                        start=True, stop=False,
                    )
                    nc.tensor.matmul(
                        ps_cov4[:, gs], sTn4[:, gs], sT4[:, gs],
                        start=False, stop=True,
                    )
                # d2 = sumsq - s^2/W (diagonal of cov), as a [1,400] row
                sq4 = work.tile([1, CHUNK], F32, tag="sq4")
                nc.vector.tensor_mul(sq4, sT4, sTn4)
                d2T4 = work.tile([1, CHUNK], F32, tag="d2T4")
                nc.vector.tensor_add(d2T4, sumsqT4, sq4)
                yield

                absC4 = work.tile([A, CHUNK], F32, tag="absC4")
                nc.scalar.activation(absC4, ps_cov4, AF.Abs)
                yield

                # outer products of the raw diagonals first; the rsqrt then
                # runs on [100,400] tiles (all partitions in parallel)
                # instead of a slow single-partition [1,400] reciprocal:
                # 1/sqrt(d_i d_j) == dinv_i dinv_j for positive d
                ps_o1 = ps.tile([A, CHUNK], F32, tag="gps")
                for g in range(GPC):
                    gs = slice(g * A, (g + 1) * A)
                    nc.tensor.matmul(
                        ps_o1[:, gs], d2T4[:, gs], d2T4[:, gs],
                        start=True, stop=True,
                    )
                yield

                invs1 = work.tile([A, CHUNK], F32, tag="invs1")
                nc.scalar.activation(invs1, ps_o1, AF.Sqrt)
                nc.vector.reciprocal(invs1, invs1)
                tmp4 = work.tile([A, CHUNK], F32, tag="tmp4")
                nc.vector.tensor_mul(tmp4, absC4, invs1)
                adj4 = work.tile([A, CHUNK], F32, tag="adj4")
                nc.gpsimd.tensor_sub(adj4, cs["eye1_4"], tmp4)
                yield

                ps_rs4 = ps.tile([1, CHUNK], F32, tag="gps")
                nc.tensor.matmul(ps_rs4, onesA, adj4, start=True, stop=True)
                yield

                rsT4 = work.tile([1, CHUNK], F32, tag="rsT4")
                nc.scalar.activation(rsT4, ps_rs4, AF.Copy)
                yield

                ps_o2 = ps.tile([A, CHUNK], F32, tag="gps")
                for g in range(GPC):
                    gs = slice(g * A, (g + 1) * A)
                    nc.tensor.matmul(
                        ps_o2[:, gs], rsT4[:, gs], rsT4[:, gs],
                        start=True, stop=True,
                    )
                yield

                # S = adj / sqrt(rs (x) rs), straight to bf16
                invs2 = work.tile([A, CHUNK], F32, tag="invs2")
                nc.scalar.activation(invs2, ps_o2, AF.Sqrt)
                nc.vector.reciprocal(invs2, invs2)
                S4 = work.tile([A, CHUNK], BF16, tag="S4")
                nc.vector.tensor_mul(S4, adj4, invs2)
                featsb4 = work.tile([A, GPC * F160], BF16, tag="featsb4")
                nc.scalar.activation(featsb4, feats4, AF.Copy)
                yield

                # layer 1: q0 = feats.T @ S ; h1 = relu(q0.T @ W1 [+ b1])
                ps_q0a = ps.tile([128, CHUNK], F32, tag="gps")
                for g in range(GPC):
                    gs = slice(g * A, (g + 1) * A)
                    nc.tensor.matmul(
                        ps_q0a[:, gs],
                        featsb4[:, g * F160:g * F160 + 128],
                        S4[:, gs], start=True, stop=True,
                    )
                ps_q0b = ps.tile([32, CHUNK], F32, tag="gps")
                for g in range(GPC):
                    gs = slice(g * A, (g + 1) * A)
                    nc.tensor.matmul(
                        ps_q0b[:, gs],
                        featsb4[:, g * F160 + 128:(g + 1) * F160],
                        S4[:, gs], start=True, stop=True,
                    )
                yield

                q0a4 = work.tile([128, CHUNK], BF16, tag="q0a4")
                nc.scalar.activation(q0a4, ps_q0a, AF.Copy)
                q0b4 = work.tile([32, CHUNK], BF16, tag="q0b4")
                nc.vector.tensor_copy(q0b4, ps_q0b)
                yield

                ps_h1 = ps.tile([A, GPC * H], F32, tag="gps")
                for g in range(GPC):
                    gh = slice(g * H, (g + 1) * H)
                    gs = slice(g * A, (g + 1) * A)
                    nc.tensor.matmul(
                        ps_h1[:, gh], q0a4[:, gs], cs["W1a"],
                        start=True, stop=False,
                    )
                    last = "b1row" not in cs
                    nc.tensor.matmul(
                        ps_h1[:, gh], q0b4[:, gs], cs["W1b"],
                        start=False, stop=last,
                    )
                    if "b1row" in cs:
                        nc.tensor.matmul(
                            ps_h1[:, gh], ones1A, cs["b1row"],
                            start=False, stop=True,
                        )
                yield

                h1_4 = work.tile([A, GPC * H], BF16, tag="h1_4")
                nc.scalar.activation(h1_4, ps_h1, AF.Relu)
                yield

                # layer 2
                ps_q1 = ps.tile([H, CHUNK], F32, tag="gps")
                for g in range(GPC):
                    gs = slice(g * A, (g + 1) * A)
                    nc.tensor.matmul(
                        ps_q1[:, gs], h1_4[:, g * H:(g + 1) * H], S4[:, gs],
                        start=True, stop=True,
                    )
                yield

                q1_4 = work.tile([H, CHUNK], BF16, tag="q1_4")
                nc.vector.tensor_copy(q1_4, ps_q1)
                yield

                ps_h2 = ps.tile([A, GPC * H], F32, tag="gps")
                for g in range(GPC):
                    gh = slice(g * H, (g + 1) * H)
                    gs = slice(g * A, (g + 1) * A)
                    last = "b2row" not in cs
                    nc.tensor.matmul(
                        ps_h2[:, gh], q1_4[:, gs], cs["W2"],
                        start=True, stop=last,
                    )
                    if "b2row" in cs:
                        nc.tensor.matmul(
                            ps_h2[:, gh], ones1A, cs["b2row"],
                            start=False, stop=True,
                        )
                yield

                h2_4 = work.tile([A, GPC * H], BF16, tag="h2_4")
                nc.scalar.activation(h2_4, ps_h2, AF.Relu)
                yield

                # layer 3 (emitted transposed): h3t = relu(W3.T @ (h2.T @ S))
                ps_q2 = ps.tile([H, CHUNK], F32, tag="gps")
                for g in range(GPC):
                    gs = slice(g * A, (g + 1) * A)
                    nc.tensor.matmul(
                        ps_q2[:, gs], h2_4[:, g * H:(g + 1) * H], S4[:, gs],
                        start=True, stop=True,
                    )
                yield

                q2_4 = work.tile([H, CHUNK], BF16, tag="q2_4")
                nc.vector.tensor_copy(q2_4, ps_q2)
                yield

                ps_h3 = ps.tile([H, CHUNK], F32, tag="gps")
                nc.tensor.matmul(ps_h3, cs["W3"], q2_4, start=True, stop=True)
                yield

                # h3t: full [128,CHUNK] tile; h3b3: base-0 copy of rows
                # 96..127 (base 96 is illegal for matmul operand APs)
                h3t, h3b3 = h3t_dst
                if "b3col" in cs:
                    nc.scalar.activation(h3t, ps_h3, AF.Relu, bias=cs["b3col"])
                    nc.vector.tensor_scalar(
                        h3b3, ps_h3[96:128, :], cs["b3col"][96:128, :], 0.0,
                        op0=mybir.AluOpType.add, op1=mybir.AluOpType.max,
                    )
                else:
                    nc.scalar.activation(h3t, ps_h3, AF.Relu)
                    nc.vector.tensor_scalar_max(h3b3, ps_h3[96:128, :], 0.0)

            def conv_chunk(ch, h3pair, gcn_iter):
                """Two 1x3 convs along the hidden axis for CHUNK nodes.
                h3pair: (h3t [128,CHUNK] bf16, h3b3 [32,CHUNK] base-0 copy
                of rows 96..127). conv1 stationaries are K=32 banded
                patterns (windows at bases 0/32/64, s=3 via h3b3); conv2
                accumulates all 128 positions into one full-bank PSUM group
                via column-padded [128,128] patterns. gcn_iter is advanced
                periodically to interleave next-chunk GCN work."""
                h3t, h3b3 = h3pair
                P = cs["cw1p"]
                po = pso.tile([H, CHUNK], F32, tag="po", name=f"po_{ch}")
                for m in range(H):
                    s, i = m // 32, m % 32
                    py = psy.tile([H, CHUNK], F32, tag="py")
                    if m <= 30:
                        # K=32 window at base 0 (h3 rows 0..31); block 30
                        # covers m==0, interior blocks i-1 cover 1..30
                        blk = 30 if m == 0 else i - 1
                        nc.tensor.matmul(
                            py, P[0:32, blk * H:(blk + 1) * H],
                            h3t[0:32, :], start=True, stop=True,
                        )
                    elif m >= 97:
                        # K=32 window on the base-0 copy of rows 96..127
                        blk = 31 if m == 127 else i - 1
                        nc.tensor.matmul(
                            py, P[0:32, blk * H:(blk + 1) * H],
                            h3b3, start=True, stop=True,
                        )
                    else:
                        # middle positions: full K=128 banded pattern
                        if m == 31:
                            lhs = cs["cw1e31"]
                        elif m == 96:
                            lhs = cs["cw1e96"]
                        else:
                            lhs = cs[f"cw1f{s}"][:, i * H:(i + 1) * H]
                        nc.tensor.matmul(py, lhs, h3t, start=True, stop=True)
                    ysb = convsb.tile([H, CHUNK], BF16, tag="ysb")
                    if "cb1col" in cs:
                        if (m % 2) == 0:
                            nc.scalar.activation(
                                ysb, py, AF.Relu, bias=cs["cb1col"]
                            )
                        else:
                            nc.vector.tensor_scalar(
                                ysb, py, cs["cb1col"], 0.0,
                                op0=mybir.AluOpType.add,
                                op1=mybir.AluOpType.max,
                            )
                    else:
                        if (m % 2) == 0:
                            nc.scalar.activation(ysb, py, AF.Relu)
                        else:
                            nc.vector.tensor_scalar_max(ysb, py, 0.0)
                    # conv2: full-bank accumulate, one group per chunk
                    nc.tensor.matmul(
                        po, cs[f"cw2q{s}"][:, i * H:(i + 1) * H], ysb,
                        start=(m == 0), stop=(m == 127),
                    )
                    if m % 6 == 5:
                        next(gcn_iter, None)
                # evict + transpose to [n, j] and store
                osb = convsb.tile([H, CHUNK], F32, tag="osb")
                nc.vector.tensor_copy(osb, po)
                for b in range(GPC):
                    ptr = pst.tile([A, H], F32, tag="ptr")
                    nc.tensor.transpose(
                        ptr, osb[:, A * b:A * (b + 1)], cs["eyeH"]
                    )
                    otr = convsb.tile([A, H], F32, tag="otr")
                    if meta["cb2"] != 0.0:
                        nc.scalar.activation(otr, ptr, AF.Copy, bias=meta["cb2"])
                    else:
                        nc.scalar.activation(otr, ptr, AF.Copy)
                    nbase = ch * CHUNK + A * b
                    nc.sync.dma_start(out=out_e[nbase:nbase + A, :], in_=otr)
                for _ in gcn_iter:
                    pass

            # chunk 0's GCN runs standalone (overlaps the const DMAs);
            # chunk ch+1's GCN interleaves into chunk ch's conv m-loop.
            _h3ctr = [0]

            def h3tiles():
                _h3ctr[0] += 1
                n = _h3ctr[0]
                return (
                    h3pool.tile([H, CHUNK], BF16, tag="h3t", name=f"h3t_{n}"),
                    h3pool.tile([32, CHUNK], BF16, tag="h3b3", name=f"h3b3_{n}"),
                )

            h3t0 = h3tiles()
            for _ in gcn_chunk(0, h3t0, (feats4_0, rT4_0)):
                pass
            prev = h3t0
            for ch in range(NCHUNK):
                if ch + 1 < NCHUNK:
                    h3tn = h3tiles()
                    nxt = gcn_chunk(ch + 1, h3tn)
                else:
                    h3tn, nxt = None, iter(())
                conv_chunk(ch, prev, nxt)
                prev = h3tn

    nc.finalize()
    return nc


_CACHE = {}


def _get_nc(consts, meta):
    key = ("nc", meta["cb2"], tuple(sorted(consts.keys())))
    if key not in _CACHE:
        _CACHE[key] = _build(consts, meta)
    return _CACHE[key]


def kernel(**inputs):
    from concourse.bass_utils import run_bass_kernel_spmd

    consts, meta = _host_consts(inputs)
    nc = _get_nc(consts, meta)
    x = np.ascontiguousarray(np.asarray(inputs["x"], np.float32))
    in_maps = []
    for c in range(NCORES):
        m = {"x": np.ascontiguousarray(x[c * NODES:(c + 1) * NODES])}
        m.update(consts)
        in_maps.append(m)
    res = run_bass_kernel_spmd(nc, in_maps, core_ids=list(range(NCORES)))
    out = np.concatenate([res.results[c]["out"] for c in range(NCORES)], axis=0)
    return out.astype(np.float32)


def run_traced(inputs, tmpdir=None):
    """For test.py: run with profiling; returns (out, BassKernelResults)."""
    from concourse.bass_utils import run_bass_kernel_spmd

    consts, meta = _host_consts(inputs)
    nc = _get_nc(consts, meta)
    x = np.ascontiguousarray(np.asarray(inputs["x"], np.float32))
    in_maps = []
    for c in range(NCORES):
        m = {"x": np.ascontiguousarray(x[c * NODES:(c + 1) * NODES])}
        m.update(consts)
        in_maps.append(m)
    res = run_bass_kernel_spmd(
        nc, in_maps, core_ids=list(range(NCORES)), trace=True, tmpdir=tmpdir
    )
    out = np.concatenate([res.results[c]["out"] for c in range(NCORES)], axis=0)
    return out.astype(np.float32), res
